# revision 53
# baseline (speedup 1.0000x reference)
"""TRN2 Bass kernel for fused MHA (softmax-over-query quirk) + out-proj + residual + LayerNorm.

Problem shapes (hardcoded): tokens [4,2048,1024], Wq/Wk [16,1024,64], Wv [16,1024,64],
Wo [1024,1024], gamma/beta [1024]. Output [4,2048,1024] fp32.

Sharding: 8 cores, core c owns (batch b=c//2, S-half jc=c%2) of the OUTPUT rows.
No collectives. Each core computes, for its batch b:
  qT[dk,i] (full S), kT[dk,j] (its half), V[i,dv] (full S) in bf16,
  scores^T[i,j] = q_i.k_j (PSUM fp32), e = exp(scores/8) (bf16),
  heads^T[dv,j] + rowsum row via a ones-column appended to V,
  multi^T = heads^T / rowsum, out = multi @ Wo + tokens, LayerNorm rows.

QKV and output projections run in fp8e4m3 with DoubleRow perf mode (2
K-planes per matmul): tokens/multi cast to fp8, weights scaled x256 (dodges
e4m3 subnormals). The x256 scale cancels exactly: scores pick up 2^16
(folded into the exp scale), heads/rowsum both pick up 2^8 (ones column =
256), and the out-proj 2^8 is divided out in the residual-add STT.
Projections are interleaved into the attention stream as hooks so the
Scalar-engine exp stream starts ~20us in instead of after all projections;
attention runs pair-major over two j-half blocks per pair (attnV
accumulators [65,512] = 2 PSUM banks, freeing a 6-deep 1-bank scores
ring: a scores matmul waits on an exp ~3 i-chunks back). Each pair's
j-half-1 block has no inherent hook needs, so it carries the next pair's
projection chains and a share of the V projections - hook load stays
near-uniform across all 256 iterations and the exp stream runs close to
its 570ns/tile floor. Scores/attnV in bf16 matmuls with fp32 PSUM;
residual + LN in fp32 (variance on alternating Scalar/DVE engines).
Measured ~380us on 8 cores (baseline 513us), rel err 7e-4.
"""

import numpy as np
import ml_dtypes

BF16 = ml_dtypes.bfloat16
FP8 = ml_dtypes.float8_e4m3

B, S, D, H, DK, DV = 4, 2048, 1024, 16, 64, 64
NCORES = 8
NPAIR = 8     # head pairs
NKC = 8       # D // 128 contraction chunks
NIC = 16      # S // 128 i-chunks
JW = 1024     # j columns per core (S/2)
NJCH = 8      # JW // 128
LN_EPS = 1e-5
WSCALE = 256.0  # fp8 weight pre-scale (power of 2)

_CACHE = {}


def _build_nc(apply_affine):
    import concourse.tile as tile
    from concourse import bacc, mybir

    F32 = mybir.dt.float32
    BF = mybir.dt.bfloat16
    F8 = mybir.dt.float8e4
    Exp = mybir.ActivationFunctionType.Exp
    Square = mybir.ActivationFunctionType.Square
    Sqrt = mybir.ActivationFunctionType.Sqrt
    mult = mybir.AluOpType.mult
    add = mybir.AluOpType.add
    DR = mybir.MatmulPerfMode.DoubleRow

    nc = bacc.Bacc(
        "TRN2",
        target_bir_lowering=False,
        debug=False,
        enable_asserts=False,
        num_devices=NCORES,
    )

    # DRAM I/O (per-core views; host prepares layouts)
    tokT_d = nc.dram_tensor("tokT", (128, NKC, S), F8, kind="ExternalInput").ap()
    tokTj_d = nc.dram_tensor("tokTj", (128, NKC, JW), F8, kind="ExternalInput").ap()
    wq_d = nc.dram_tensor("wq", (128, NKC, H * DK), F8, kind="ExternalInput").ap()
    wk_d = nc.dram_tensor("wk", (128, NKC, H * DK), F8, kind="ExternalInput").ap()
    wv_d = nc.dram_tensor("wv", (128, NKC, H * DV), F8, kind="ExternalInput").ap()
    wo_d = nc.dram_tensor("wo", (128, NKC, D), F8, kind="ExternalInput").ap()
    tokres_d = nc.dram_tensor("tokres", (128, NJCH, D), F32, kind="ExternalInput").ap()
    if apply_affine:
        gamma_d = nc.dram_tensor("gamma_bc", (128, D), F32, kind="ExternalInput").ap()
        beta_d = nc.dram_tensor("beta_bc", (128, D), F32, kind="ExternalInput").ap()
    out_d = nc.dram_tensor("out", (128, NJCH, D), F32, kind="ExternalOutput").ap()
    from contextlib import ExitStack

    from concourse.bass import _add_dep_helper

    # Chain all PE matmuls in emission order: stops the scheduler from
    # interleaving row-conflicting matmuls and keeps the stream dense.
    _prev_mm = [None]

    def mm(*args, **kwargs):
        inst = nc.tensor.matmul(*args, **kwargs)
        if _prev_mm[0] is not None:
            _add_dep_helper(inst.ins, _prev_mm[0].ins, sync=False, reason="pe-order")
        _prev_mm[0] = inst
        return inst

    with tile.TileContext(nc) as tc, ExitStack() as stack:
        persist = stack.enter_context(tc.tile_pool(name="persist", bufs=1))
        qT_sb = persist.tile([128, NPAIR, S], BF)          # [pair-dk, pr, i]
        kT_sb = persist.tile([128, NPAIR, JW], BF)         # [pair-dk, pr, j]
        v_sb = persist.tile([128, NIC, H, DV + 1], BF)     # [i%128, ic, h, dv|256s]
        # multi^T in fp8, one tile per KC-PAIR ([128, 2, JW], middle dim = the
        # DoubleRow K-plane) so the out-proj runs fp8 DoubleRow; per-pair-ish
        # tiles keep out-proj dep tracking from serializing on the last write
        multiT = [
            persist.tile([128, 2, JW], F8, name=f"multiT{i}") for i in range(NKC // 2)
        ]
        eps_sb = persist.tile([128, 1], F32)
        warm_w = persist.tile([1, 512], BF)
        nc.vector.memset(warm_w[:], 0.0)
        if apply_affine:
            gamma_sb = persist.tile([128, D], F32)
            beta_sb = persist.tile([128, D], F32)
            nc.sync.dma_start(gamma_sb[:], gamma_d[:])
            nc.sync.dma_start(beta_sb[:], beta_d[:])
        nc.vector.memset(eps_sb[:], LN_EPS)
        for ic in range(NIC):
            # ones column scaled by WSCALE so rowsum matches the x256 V scale
            nc.vector.memset(v_sb[:, ic, :, DV : DV + 1], WSCALE)

        # pools that outlive pa must be allocated first (LIFO release)
        # psS: six 1-bank [128,512] slots (the j-half sweeps shrink the attnV
        # accumulators to 2 banks, freeing 2 more for scores ring depth).
        psS = tc.alloc_tile_pool(name="psS", bufs=6, space="PSUM")
        psAcc = tc.alloc_tile_pool(name="psAcc", bufs=2, space="PSUM")
        pe_pool = stack.enter_context(tc.tile_pool(name="pe", bufs=16))
        pn_pool = stack.enter_context(tc.tile_pool(name="pn", bufs=2))
        pdram = stack.enter_context(tc.tile_pool(name="pdram", bufs=2, space="DRAM"))

        # tokTj and wk outlive pa (the k j-half-1 chains consume them in
        # sweep 1), so they live in their own right-side pool.
        paJ = tc.alloc_tile_pool(name="paJ", bufs=1, side="right")
        tokTj_sb = paJ.tile([128, NKC, JW], F8)
        wk_sb = paJ.tile([128, NKC, H * DK], F8)
        pa = tc.alloc_tile_pool(name="pa", bufs=1)
        wq_sb = pa.tile([128, NKC, H * DK], F8)
        tokT_sb = pa.tile([128, NKC, S], F8)
        wv_sb = pa.tile([128, NKC, H * DV], F8)

        # Startup DMA over both HWDGE rings (SP + ACT, independent FIFOs).
        # Critical path to the first exp is only the pair-0 weight slices +
        # tokens-i0 + tokTj; everything else queues behind.
        nc.sync.dma_start(wq_sb[:, :, 0:128], wq_d[:, :, 0:128])
        nc.sync.dma_start(wk_sb[:, :, 0:128], wk_d[:, :, 0:128])
        # tokens i 0..511 first (q chain t0, V chains), as single 3D transfers
        nc.scalar.dma_start(tokT_sb[:, 0:4, 0:512], tokT_d[:, 0:4, 0:512])
        nc.scalar.dma_start(tokT_sb[:, 4:8, 0:512], tokT_d[:, 4:8, 0:512])
        nc.sync.dma_start(tokTj_sb[:, 0:4], tokTj_d[:, 0:4])
        nc.sync.dma_start(tokTj_sb[:, 4:8], tokTj_d[:, 4:8])
        nc.sync.dma_start(tokT_sb[:, :, 512:1024], tokT_d[:, :, 512:1024])
        nc.scalar.dma_start(wv_sb[:], wv_d[:])
        nc.sync.dma_start(tokT_sb[:, :, 1024:2048], tokT_d[:, :, 1024:2048])
        nc.sync.dma_start(wq_sb[:, :, 128:], wq_d[:, :, 128:])
        nc.sync.dma_start(wk_sb[:, :, 128:], wk_d[:, :, 128:])

        def proj_chain(pr, which, t):
            """One 512-wide fp8 DoubleRow projection chain via a borrowed
            scores-pool slot."""
            w_sb, dst, rhs_sb = (
                (wq_sb, qT_sb, tokT_sb) if which == "q" else (wk_sb, kT_sb, tokTj_sb)
            )
            ps = psS.tile([128, 512], F32, tag="sc", name=f"pj{which}{pr}_{t}")
            for kc in range(0, NKC, 2):
                mm(
                    ps[:],
                    w_sb[:, kc : kc + 2, pr * 128 : (pr + 1) * 128],
                    rhs_sb[:, kc : kc + 2, t * 512 : (t + 1) * 512],
                    start=(kc == 0),
                    stop=(kc == NKC - 2),
                    perf_mode=DR,
                )
            nc.vector.tensor_copy(out=dst[:, pr, t * 512 : (t + 1) * 512], in_=ps[:])

        def proj_v(ic, nb):
            """fp8 DoubleRow V projection for one i-chunk and one 8-head half
            via a borrowed scores-pool slot."""
            for nb in (nb,):
                ps = psS.tile([128, 512], F32, tag="sc", name=f"pjv{ic}_{nb}")
                for kc in range(0, NKC, 2):
                    mm(
                        ps[:],
                        tokT_sb[:, kc : kc + 2, ic * 128 : (ic + 1) * 128],
                        wv_sb[:, kc : kc + 2, nb * 512 : (nb + 1) * 512],
                        start=(kc == 0),
                        stop=(kc == NKC - 2),
                        perf_mode=DR,
                    )
                nc.vector.tensor_copy(
                    out=v_sb[:, ic, nb * 8 : (nb + 1) * 8, 0:DV],
                    in_=ps.rearrange("p (h v) -> p h v", h=8),
                )

        def normalize(pr, acc, sweep):
            """multi^T[h] = heads^T / rowsum for one j-half; DVE/DMA only."""
            j0 = sweep * 512
            for hh in range(2):
                h = 2 * pr + hh
                hraw = pn_pool.tile(
                    [DV + 1, 512], F32, tag="hraw", name=f"hraw{sweep}_{h}"
                )
                nc.vector.tensor_copy(out=hraw[:], in_=acc[hh][:])  # frees acc
                rs_dram = pdram.tile([1, 512], F32, tag="rsd", name=f"rsd{sweep}_{h}")
                nc.sync.dma_start(out=rs_dram[:], in_=hraw[DV : DV + 1, :])
                rec_in = pn_pool.tile([DV, 512], F32, tag="rin", name=f"rin{sweep}_{h}")
                nc.gpsimd.dma_start(out=rec_in[:], in_=rs_dram.to_broadcast((DV, 512)))
                nc.vector.reciprocal_approx_fast(out=rec_in[:], in_=rec_in[:])
                if hh == 0:
                    nc.vector.tensor_tensor(
                        multiT[pr // 2][0:64, pr % 2, j0 : j0 + 512],
                        hraw[0:DV, :], rec_in[:], mult,
                    )
                else:
                    tmp64 = pn_pool.tile(
                        [DV, 512], F8, tag="tmp64", name=f"tmp{sweep}_{h}"
                    )
                    nc.vector.tensor_tensor(tmp64[:], hraw[0:DV, :], rec_in[:], mult)
                    nc.sync.dma_start(
                        out=multiT[pr // 2][64:128, pr % 2, j0 : j0 + 512],
                        in_=tmp64[:],
                    )

        def attention(hooks_by_sweep, after_pair=None):
            """Two j-half sweeps over all pairs. Halving the attnV accumulators
            to [65,512] (1 bank each, 2 live) frees PSUM for a 6-deep scores
            ring: a scores matmul waits on an exp from ~3 i-chunks back, so
            projection hooks and LDW/copy latencies no longer starve the exp
            stream. Hooks and the lag-3 attnV are emitted BEFORE each
            iteration's scores to fill that wait. hooks_by_sweep[s][pr][ic]
            is a list of thunks."""
            from collections import deque

            pending = deque()   # (eTs, ic, pr, acc, sweep), newest at right

            def do_attnv(peT, pic, ppr, pacc, psweep):
                for hh in range(2):
                    mm(
                        pacc[hh][:],
                        v_sb[:, pic, 2 * ppr + hh, :],
                        peT[hh][:],
                        start=(pic == 0),
                        stop=(pic == NIC - 1),
                    )

            for pr in range(NPAIR):
                for sweep in range(2):
                    j0 = sweep * 512
                    acc = [
                        psAcc.tile(
                            [DV + 1, 512], F32, tag="acc", name=f"acc{sweep}_{pr}_{hh}"
                        )
                        for hh in range(2)
                    ]
                    hooks = hooks_by_sweep.get((pr, sweep), {})
                    # pair 0 runs a deeper attnV lag so its second j-half can
                    # carry the V projections for i-chunks 9-15, each hooked
                    # two iterations before its attnV pops
                    limit = 7 if pr == 0 else 3
                    for ic in range(NIC):
                        for fn in hooks.get(ic, ()):
                            fn()
                        while len(pending) >= limit:
                            pa = pending.popleft()
                            do_attnv(*pa)
                            if pa[1] == NIC - 1:
                                normalize(pa[2], pa[3], pa[4])
                        # scores^T, row-tiled pair (K=64 at partitions 0/64);
                        # one 1-bank PSUM slot and one exp per head
                        eTs = []
                        for hh in range(2):
                            ps_s = psS.tile(
                                [128, 512], F32, tag="sc",
                                name=f"ps_s{sweep}_{pr}_{ic}_{hh}",
                            )
                            mm(
                                ps_s[:],
                                qT_sb[hh * 64 : (hh + 1) * 64, pr, ic * 128 : (ic + 1) * 128],
                                kT_sb[hh * 64 : (hh + 1) * 64, pr, j0 : j0 + 512],
                                start=True,
                                stop=True,
                            )
                            eT = pe_pool.tile(
                                [128, 512], BF, tag="eT",
                                name=f"eT{sweep}_{pr}_{ic}_{hh}",
                            )
                            # x256-scaled q and k: fold 2^-16 into the exp scale
                            nc.scalar.activation(
                                eT[:], ps_s[:], Exp, scale=0.125 / (WSCALE * WSCALE)
                            )
                            eTs.append(eT)
                        pending.append((eTs, ic, pr, acc, sweep))
                    if after_pair and (sweep, pr) in after_pair:
                        after_pair[(sweep, pr)]()
            while pending:
                pa = pending.popleft()
                do_attnv(*pa)
                if pa[1] == NIC - 1:
                    normalize(pa[2], pa[3], pa[4])

        # Hook schedule for pair-major order (pair p: j-half-0 block then
        # j-half-1 block). Each pair's j-half-1 block has no inherent hook
        # needs, so it carries the NEXT pair's q t0 / k chains and a share of
        # the deferred V projections — load stays near-uniform across all 256
        # iterations instead of crowding a single global sweep.
        hooks = {}

        def add_hook(pr, sweep, ic, fn):
            hooks.setdefault((pr, sweep), {}).setdefault(ic, []).append(fn)

        # V heads 0-7 (pairs 0-3): i-chunks 0-9 in pair 0's first block, the
        # lag-6 tail (i-chunks 10-15) in its second block just before each is
        # consumed. V heads 8-15 (first used at pair 4) spread over pairs
        # 1-3's second blocks.
        add_hook(0, 0, 1, lambda: proj_v(0, 0))
        add_hook(0, 0, 1, lambda: proj_v(1, 0))
        for ic in range(2, 9):
            add_hook(0, 0, ic, lambda ic=ic: proj_v(ic, 0))
        add_hook(0, 0, 14, lambda: proj_v(9, 0))
        add_hook(0, 0, 15, lambda: proj_v(10, 0))
        for k in range(5):
            add_hook(0, 1, k, lambda k=k: proj_v(11 + k, 0))
        for k, ic in enumerate((1, 3, 5, 13, 14, 15)):
            add_hook(1, 1, ic, lambda k=k: proj_v(k, 1))
        for k, ic in enumerate((1, 2, 3, 13, 14)):
            add_hook(2, 1, ic, lambda k=k: proj_v(6 + k, 1))
        for k, ic in enumerate((1, 2, 3)):
            add_hook(3, 1, ic, lambda k=k: proj_v(11 + k, 1))
        add_hook(4, 0, 12, lambda: proj_v(14, 1))
        add_hook(4, 0, 13, lambda: proj_v(15, 1))
        for pr in range(NPAIR):
            # own q t1-3, consumed by this pair's first block from ic4/8/12 on
            for t, ic in ((1, 3), (2, 7), (3, 11)):
                add_hook(pr, 0, ic, lambda pr=pr, t=t: proj_chain(pr, "q", t))
        add_hook(0, 0, 5, lambda: proj_chain(0, "k", 1))
        for pr in range(7):  # next pair's bootstrap rides the j-half-1 block
            add_hook(pr, 1, 7, lambda pr=pr: proj_chain(pr + 1, "q", 0))
            add_hook(pr, 1, 9, lambda pr=pr: proj_chain(pr + 1, "k", 0))
            add_hook(pr, 1, 11, lambda pr=pr: proj_chain(pr + 1, "k", 1))

        pc_tiles = {}

        def open_phase_c():
            # pa's tensors are all dead once sweep 0 ends; reuse the space for
            # phase C inputs so their DMA overlaps all of sweep 1.
            pa.release()
            pc = stack.enter_context(tc.tile_pool(name="pc", bufs=1))
            pc_tiles["wo"] = pc.tile([128, NKC, D], F8, name="wo_sb")
            pc_tiles["tokres"] = pc.tile([128, NJCH, D], F32, name="tokres_sb")
            nc.sync.dma_start(pc_tiles["wo"][:], wo_d[:])
            nc.sync.dma_start(pc_tiles["tokres"][:], tokres_d[:])

        # Warm-up during the startup DMA wait: ~5us of dummy matmuls ramp the
        # PE clock (HAM gate opens after ~3us busy) and one dummy exp pulls
        # the ACT table load off the first real exp's critical path.
        ps_warm = psS.tile([DV, 512], F32, tag="sc", name="ps_warm")
        for r in range(16):
            mm(ps_warm[:], warm_w[0:1, 0:DV], warm_w[:], start=True, stop=True)
        warm_eT = pn_pool.tile([DV, 512], BF, tag="hraw", name="warm_eT")
        nc.scalar.activation(warm_eT[:], ps_warm[:], Exp, scale=1.0)

        # upfront: just enough projection for sweep-0 pair-0's first scores
        proj_chain(0, "q", 0)
        proj_chain(0, "k", 0)

        attention(hooks, after_pair={(0, 7): open_phase_c})
        wo_sb = pc_tiles["wo"]
        tokres_sb = pc_tiles["tokres"]
        paJ.release()
        psAcc.release()
        psS.release()
        # ---------------- Phase C: out-proj + residual + LayerNorm ----------------
        with (
            tc.tile_pool(name="pC", bufs=4) as pC,
            tc.tile_pool(name="pStats", bufs=8) as pStats,
            tc.tile_pool(name="psC", bufs=4, space="PSUM") as psC,
        ):
            # Out-proj in two steps per jch: kc 0-6 accumulate early (their
            # multiT chunks are ready pairs before the last normalize), kc 7
            # finishes when multiT[7] lands. Prefilling 4 PSUM groups hides
            # the last normalize's DRAM round-trip behind ~12us of matmuls.
            prefill = {}

            def emit_prefill(jch):
                ps_o = psC.tile([128, D], F32, tag="po", name=f"ps_o{jch}")
                for kcp in range(NKC // 2 - 1):
                    lhsT = multiT[kcp][:, :, jch * 128 : (jch + 1) * 128]
                    for nb in range(2):
                        mm(
                            ps_o[:, nb * 512 : (nb + 1) * 512],
                            lhsT,
                            wo_sb[:, 2 * kcp : 2 * kcp + 2, nb * 512 : (nb + 1) * 512],
                            start=(kcp == 0),
                            stop=False,
                            perf_mode=DR,
                        )
                prefill[jch] = ps_o

            for jch in range(4):
                emit_prefill(jch)
            for jch in range(NJCH):
                ps_o = prefill.pop(jch)
                lhsT = multiT[NKC // 2 - 1][:, :, jch * 128 : (jch + 1) * 128]
                for nb in range(2):
                    mm(
                        ps_o[:, nb * 512 : (nb + 1) * 512],
                        lhsT,
                        wo_sb[:, NKC - 2 : NKC, nb * 512 : (nb + 1) * 512],
                        start=False,
                        stop=True,
                        perf_mode=DR,
                    )
                # x = psum + residual, sum_t = rowsum(x), in one DVE pass
                x_sb = pC.tile([128, D], F32, tag="x", name=f"x{jch}")
                sum_t = pStats.tile([128, 1], F32, tag="sum", name=f"sum{jch}")
                # x = psum/WSCALE + residual (undo the Wo fp8 pre-scale),
                # sum_t = rowsum(x), in one DVE pass
                nc.vector.scalar_tensor_tensor(
                    out=x_sb[:],
                    in0=ps_o[:],
                    scalar=1.0 / WSCALE,
                    in1=tokres_sb[:, jch, :],
                    op0=mult,
                    op1=add,
                    accum_out=sum_t[:],
                )
                negmean = pStats.tile([128, 1], F32, tag="nm", name=f"nm{jch}")
                nc.vector.tensor_scalar_mul(negmean[:], sum_t[:], -1.0 / D)
                # Variance: alternate engines by jch parity so the serialized
                # LN tail splits between Scalar (sum((x-m)^2) via Square) and
                # DVE (sum(x^2)/D - mean^2 via an STT x*x pass).
                scrap = pC.tile([128, D], BF, tag="scrap", name=f"scrap{jch}")
                ssq = pStats.tile([128, 1], F32, tag="ssq", name=f"ssq{jch}")
                if jch % 2 == 0:
                    nc.scalar.activation(
                        scrap[:], x_sb[:], Square, bias=negmean[:], accum_out=ssq[:]
                    )
                else:
                    nc.vector.scalar_tensor_tensor(
                        out=scrap[:], in0=x_sb[:], scalar=0.0, in1=x_sb[:],
                        op0=add, op1=mult, accum_out=ssq[:],
                    )
                    m2 = pStats.tile([128, 1], F32, tag="m2", name=f"m2{jch}")
                    nc.vector.tensor_tensor(m2[:], negmean[:], negmean[:], mult)
                    nc.vector.tensor_scalar_mul(m2[:], m2[:], -float(D))
                    nc.vector.tensor_tensor(ssq[:], ssq[:], m2[:], add)
                std_t = pStats.tile([128, 1], F32, tag="std", name=f"std{jch}")
                nc.scalar.activation(std_t[:], ssq[:], Sqrt, bias=eps_sb[:], scale=1.0 / D)
                rstd = pStats.tile([128, 1], F32, tag="rstd", name=f"rstd{jch}")
                nc.vector.reciprocal(rstd[:], std_t[:])
                # (x - m) * rstd == x*rstd + (negmean*rstd), one ACT op
                rstd_nm = pStats.tile([128, 1], F32, tag="rnm", name=f"rnm{jch}")
                nc.vector.tensor_tensor(rstd_nm[:], negmean[:], rstd[:], mult)
                out_sb = pC.tile([128, D], F32, tag="out", name=f"out{jch}")
                nc.scalar.activation(
                    out_sb[:],
                    x_sb[:],
                    mybir.ActivationFunctionType.Identity,
                    bias=rstd_nm[:],
                    scale=rstd[:],
                )
                if apply_affine:
                    nc.gpsimd.tensor_tensor(out_sb[:], out_sb[:], gamma_sb[:], mult)
                    nc.gpsimd.tensor_tensor(out_sb[:], out_sb[:], beta_sb[:], add)
                nc.sync.dma_start(out_d[:, jch], out_sb[:])
                # second prefill wave once the first four STTs are emitted, so
                # the PE chain never parks on a not-yet-freed PSUM group
                if jch == 3:
                    for j2 in range(4, NJCH):
                        emit_prefill(j2)

    nc.compile()
    return nc


def _prep_inputs(tokens, Wq, Wk, Wv, Wo, gamma, beta):
    """Host-side layout prep. Returns per-core input maps."""
    tokens = np.ascontiguousarray(np.asarray(tokens, dtype=np.float32))
    # weights -> [p, kc, n] with row index kc*128+p
    def rows128(a):  # [1024, N] -> [128, 8, N]
        return np.ascontiguousarray(
            a.reshape(NKC, 128, a.shape[-1]).transpose(1, 0, 2)
        )

    wq_all = rows128(
        (np.asarray(Wq).transpose(1, 0, 2).reshape(D, H * DK) * WSCALE).astype(FP8)
    )
    wk_all = rows128(
        (np.asarray(Wk).transpose(1, 0, 2).reshape(D, H * DK) * WSCALE).astype(FP8)
    )
    wv_all = rows128(
        (np.asarray(Wv).transpose(1, 0, 2).reshape(D, H * DV) * WSCALE).astype(FP8)
    )
    wo_all = rows128((np.asarray(Wo) * WSCALE).astype(FP8))
    gamma_bc = np.ascontiguousarray(
        np.broadcast_to(np.asarray(gamma, np.float32), (128, D))
    )
    beta_bc = np.ascontiguousarray(
        np.broadcast_to(np.asarray(beta, np.float32), (128, D))
    )

    tokT_by_b = []
    for b in range(B):
        tokT_by_b.append(rows128(tokens[b].T.astype(FP8)))  # [128, 8, 2048]

    in_maps = []
    for c in range(NCORES):
        b, jc = c // 2, c % 2
        tokT = tokT_by_b[b]
        tokTj = np.ascontiguousarray(tokT[:, :, jc * JW : (jc + 1) * JW])
        tokres = np.ascontiguousarray(
            tokens[b, jc * JW : (jc + 1) * JW]
            .reshape(NJCH, 128, D)
            .transpose(1, 0, 2)
        )
        in_maps.append(
            {
                "tokT": tokT,
                "tokTj": tokTj,
                "wq": wq_all,
                "wk": wk_all,
                "wv": wv_all,
                "wo": wo_all,
                "tokres": tokres,
                "gamma_bc": gamma_bc,
                "beta_bc": beta_bc,
            }
        )
    return in_maps


def run(inputs, trace=False, tmpdir=None):
    """Run on hardware; returns (output, BassKernelResults)."""
    from concourse.bass_utils import run_bass_kernel_spmd

    apply_affine = not (
        np.all(np.asarray(inputs["gamma"]) == 1.0)
        and np.all(np.asarray(inputs["beta"]) == 0.0)
    )
    key = ("nc", apply_affine)
    if key not in _CACHE:
        _CACHE[key] = _build_nc(apply_affine)
    nc = _CACHE[key]
    in_maps = _prep_inputs(**inputs)
    res = run_bass_kernel_spmd(
        nc, in_maps, core_ids=list(range(NCORES)), trace=trace, tmpdir=tmpdir
    )
    out = np.empty((B, S, D), np.float32)
    for c in range(NCORES):
        b, jc = c // 2, c % 2
        o = res.results[c]["out"]  # [128, 8, 1024]
        out[b, jc * JW : (jc + 1) * JW] = (
            o.transpose(1, 0, 2).reshape(JW, D)
        )
    return out, res


def kernel(tokens, Wq, Wk, Wv, Wo, gamma, beta):
    out, _ = run(
        dict(tokens=tokens, Wq=Wq, Wk=Wk, Wv=Wv, Wo=Wo, gamma=gamma, beta=beta)
    )
    return out


# revision 54
# speedup vs baseline: 1.1897x; 1.1897x over previous
"""TRN2 Bass kernel for fused MHA (softmax-over-query quirk) + out-proj + residual + LayerNorm.

Problem shapes (hardcoded): tokens [4,2048,1024], Wq/Wk [16,1024,64], Wv [16,1024,64],
Wo [1024,1024], gamma/beta [1024]. Output [4,2048,1024] fp32.

Sharding: 8 cores, core c owns (batch b=c//2, S-half jc=c%2) of the OUTPUT rows.
No collectives. Each core computes, for its batch b:
  qT[dk,i] (full S), kT[dk,j] (its half), V[i,dv] (full S) in bf16,
  scores^T[i,j] = q_i.k_j (PSUM fp32), e = exp(scores/8) (bf16),
  heads^T[dv,j] + rowsum row via a ones-column appended to V,
  multi^T = heads^T / rowsum, out = multi @ Wo + tokens, LayerNorm rows.

QKV and output projections run in fp8e4m3 with DoubleRow perf mode (2
K-planes per matmul): tokens/multi cast to fp8, weights scaled x256 (dodges
e4m3 subnormals). The x256 scale cancels exactly: scores pick up 2^16
(folded into the exp scale), heads/rowsum both pick up 2^8 (ones column =
256), and the out-proj 2^8 is divided out in the residual-add STT.
Projections are interleaved into the attention stream as hooks so the
Scalar-engine exp stream starts ~20us in instead of after all projections;
attention runs pair-major over two j-half blocks per pair (attnV
accumulators [65,512] = 2 PSUM banks, freeing a 6-deep 1-bank scores
ring: a scores matmul waits on an exp ~3 i-chunks back). Each pair's
j-half-1 block has no inherent hook needs, so it carries the next pair's
projection chains and a share of the V projections - hook load stays
near-uniform across all 256 iterations and the exp stream runs close to
its 570ns/tile floor. Scores/attnV in bf16 matmuls with fp32 PSUM;
residual + LN in fp32 (variance on alternating Scalar/DVE engines).
Measured ~380us on 8 cores (baseline 513us), rel err 7e-4.
"""

import numpy as np
import ml_dtypes

BF16 = ml_dtypes.bfloat16
FP8 = ml_dtypes.float8_e4m3

B, S, D, H, DK, DV = 4, 2048, 1024, 16, 64, 64
NCORES = 8
NPAIR = 8     # head pairs
NKC = 8       # D // 128 contraction chunks
NIC = 16      # S // 128 i-chunks
JW = 1024     # j columns per core (S/2)
NJCH = 8      # JW // 128
LN_EPS = 1e-5
WSCALE = 256.0  # fp8 weight pre-scale (power of 2)

_CACHE = {}


def _build_nc(apply_affine):
    import concourse.tile as tile
    from concourse import bacc, mybir

    F32 = mybir.dt.float32
    BF = mybir.dt.bfloat16
    F8 = mybir.dt.float8e4
    Exp = mybir.ActivationFunctionType.Exp
    Square = mybir.ActivationFunctionType.Square
    Sqrt = mybir.ActivationFunctionType.Sqrt
    mult = mybir.AluOpType.mult
    add = mybir.AluOpType.add
    DR = mybir.MatmulPerfMode.DoubleRow

    nc = bacc.Bacc(
        "TRN2",
        target_bir_lowering=False,
        debug=False,
        enable_asserts=False,
        num_devices=NCORES,
    )

    # DRAM I/O (per-core views; host prepares layouts)
    tokT_d = nc.dram_tensor("tokT", (128, NKC, S), F8, kind="ExternalInput").ap()
    tokTj_d = nc.dram_tensor("tokTj", (128, NKC, JW), F8, kind="ExternalInput").ap()
    wq_d = nc.dram_tensor("wq", (128, NKC, H * DK), F8, kind="ExternalInput").ap()
    wk_d = nc.dram_tensor("wk", (128, NKC, H * DK), F8, kind="ExternalInput").ap()
    wv_d = nc.dram_tensor("wv", (128, NKC, H * DV), F8, kind="ExternalInput").ap()
    wo_d = nc.dram_tensor("wo", (128, NKC, D), F8, kind="ExternalInput").ap()
    tokres_d = nc.dram_tensor("tokres", (128, NJCH, D), F32, kind="ExternalInput").ap()
    if apply_affine:
        gamma_d = nc.dram_tensor("gamma_bc", (128, D), F32, kind="ExternalInput").ap()
        beta_d = nc.dram_tensor("beta_bc", (128, D), F32, kind="ExternalInput").ap()
    out_d = nc.dram_tensor("out", (128, NJCH, D), F32, kind="ExternalOutput").ap()
    from contextlib import ExitStack

    from concourse.bass import _add_dep_helper

    # Chain all PE matmuls in emission order: stops the scheduler from
    # interleaving row-conflicting matmuls and keeps the stream dense.
    _prev_mm = [None]

    def mm(*args, **kwargs):
        inst = nc.tensor.matmul(*args, **kwargs)
        if _prev_mm[0] is not None:
            _add_dep_helper(inst.ins, _prev_mm[0].ins, sync=False, reason="pe-order")
        _prev_mm[0] = inst
        return inst

    with tile.TileContext(nc) as tc, ExitStack() as stack:
        persist = stack.enter_context(tc.tile_pool(name="persist", bufs=1))
        qT_sb = persist.tile([128, NPAIR, S], BF)          # [pair-dk, pr, i]
        kT_sb = persist.tile([128, NPAIR, JW], BF)         # [pair-dk, pr, j]
        v_sb = persist.tile([128, NIC, H, DV + 1], BF)     # [i%128, ic, h, dv|256s]
        # multi^T in fp8, one tile per KC-PAIR ([128, 2, JW], middle dim = the
        # DoubleRow K-plane) so the out-proj runs fp8 DoubleRow; per-pair-ish
        # tiles keep out-proj dep tracking from serializing on the last write
        multiT = [
            persist.tile([128, 2, JW], F8, name=f"multiT{i}") for i in range(NKC // 2)
        ]
        eps_sb = persist.tile([128, 1], F32)
        if apply_affine:
            gamma_sb = persist.tile([128, D], F32)
            beta_sb = persist.tile([128, D], F32)
            nc.sync.dma_start(gamma_sb[:], gamma_d[:])
            nc.sync.dma_start(beta_sb[:], beta_d[:])
        nc.vector.memset(eps_sb[:], LN_EPS)
        for ic in range(NIC):
            # ones column scaled by WSCALE so rowsum matches the x256 V scale
            nc.vector.memset(v_sb[:, ic, :, DV : DV + 1], WSCALE)

        # pools that outlive pa must be allocated first (LIFO release)
        # psS: six 1-bank [128,512] slots (the j-half sweeps shrink the attnV
        # accumulators to 2 banks, freeing 2 more for scores ring depth).
        psS = tc.alloc_tile_pool(name="psS", bufs=6, space="PSUM")
        psAcc = tc.alloc_tile_pool(name="psAcc", bufs=2, space="PSUM")
        pe_pool = stack.enter_context(tc.tile_pool(name="pe", bufs=16))
        pn_pool = stack.enter_context(tc.tile_pool(name="pn", bufs=2))
        pdram = stack.enter_context(tc.tile_pool(name="pdram", bufs=2, space="DRAM"))

        # tokTj and wk outlive pa (the k j-half-1 chains consume them in
        # sweep 1), so they live in their own right-side pool.
        paJ = tc.alloc_tile_pool(name="paJ", bufs=1, side="right")
        tokTj_sb = paJ.tile([128, NKC, JW], F8)
        wk_sb = paJ.tile([128, NKC, H * DK], F8)
        pa = tc.alloc_tile_pool(name="pa", bufs=1)
        wq_sb = pa.tile([128, NKC, H * DK], F8)
        tokT_sb = pa.tile([128, NKC, S], F8)
        wv_sb = pa.tile([128, NKC, H * DV], F8)

        # Startup DMA over both HWDGE rings (SP + ACT, independent FIFOs).
        # Critical path to the first exp is only the pair-0 weight slices +
        # tokens-i0 + tokTj; everything else queues behind.
        nc.sync.dma_start(wq_sb[:, :, 0:128], wq_d[:, :, 0:128])
        nc.sync.dma_start(wk_sb[:, :, 0:128], wk_d[:, :, 0:128])
        # tokens i 0..511 first (q chain t0, V chains), as single 3D transfers
        nc.scalar.dma_start(tokT_sb[:, 0:4, 0:512], tokT_d[:, 0:4, 0:512])
        nc.scalar.dma_start(tokT_sb[:, 4:8, 0:512], tokT_d[:, 4:8, 0:512])
        nc.sync.dma_start(tokTj_sb[:, 0:4], tokTj_d[:, 0:4])
        nc.sync.dma_start(tokTj_sb[:, 4:8], tokTj_d[:, 4:8])
        nc.sync.dma_start(tokT_sb[:, :, 512:1024], tokT_d[:, :, 512:1024])
        nc.scalar.dma_start(wv_sb[:], wv_d[:])
        nc.sync.dma_start(tokT_sb[:, :, 1024:2048], tokT_d[:, :, 1024:2048])
        nc.sync.dma_start(wq_sb[:, :, 128:], wq_d[:, :, 128:])
        nc.sync.dma_start(wk_sb[:, :, 128:], wk_d[:, :, 128:])

        def proj_chain(pr, which, t):
            """One 512-wide fp8 DoubleRow projection chain via a borrowed
            scores-pool slot."""
            w_sb, dst, rhs_sb = (
                (wq_sb, qT_sb, tokT_sb) if which == "q" else (wk_sb, kT_sb, tokTj_sb)
            )
            ps = psS.tile([128, 512], F32, tag="sc", name=f"pj{which}{pr}_{t}")
            for kc in range(0, NKC, 2):
                mm(
                    ps[:],
                    w_sb[:, kc : kc + 2, pr * 128 : (pr + 1) * 128],
                    rhs_sb[:, kc : kc + 2, t * 512 : (t + 1) * 512],
                    start=(kc == 0),
                    stop=(kc == NKC - 2),
                    perf_mode=DR,
                )
            nc.vector.tensor_copy(out=dst[:, pr, t * 512 : (t + 1) * 512], in_=ps[:])

        def proj_v(ic, nb):
            """fp8 DoubleRow V projection for one i-chunk and one 8-head half
            via a borrowed scores-pool slot."""
            for nb in (nb,):
                ps = psS.tile([128, 512], F32, tag="sc", name=f"pjv{ic}_{nb}")
                for kc in range(0, NKC, 2):
                    mm(
                        ps[:],
                        tokT_sb[:, kc : kc + 2, ic * 128 : (ic + 1) * 128],
                        wv_sb[:, kc : kc + 2, nb * 512 : (nb + 1) * 512],
                        start=(kc == 0),
                        stop=(kc == NKC - 2),
                        perf_mode=DR,
                    )
                nc.vector.tensor_copy(
                    out=v_sb[:, ic, nb * 8 : (nb + 1) * 8, 0:DV],
                    in_=ps.rearrange("p (h v) -> p h v", h=8),
                )

        def normalize(pr, acc, sweep):
            """multi^T[h] = heads^T / rowsum for one j-half; DVE/DMA only."""
            j0 = sweep * 512
            for hh in range(2):
                h = 2 * pr + hh
                hraw = pn_pool.tile(
                    [DV + 1, 512], F32, tag="hraw", name=f"hraw{sweep}_{h}"
                )
                nc.vector.tensor_copy(out=hraw[:], in_=acc[hh][:])  # frees acc
                rs_dram = pdram.tile([1, 512], F32, tag="rsd", name=f"rsd{sweep}_{h}")
                nc.sync.dma_start(out=rs_dram[:], in_=hraw[DV : DV + 1, :])
                rec_in = pn_pool.tile([DV, 512], F32, tag="rin", name=f"rin{sweep}_{h}")
                nc.gpsimd.dma_start(out=rec_in[:], in_=rs_dram.to_broadcast((DV, 512)))
                nc.vector.reciprocal_approx_fast(out=rec_in[:], in_=rec_in[:])
                if hh == 0:
                    nc.vector.tensor_tensor(
                        multiT[pr // 2][0:64, pr % 2, j0 : j0 + 512],
                        hraw[0:DV, :], rec_in[:], mult,
                    )
                else:
                    tmp64 = pn_pool.tile(
                        [DV, 512], F8, tag="tmp64", name=f"tmp{sweep}_{h}"
                    )
                    nc.vector.tensor_tensor(tmp64[:], hraw[0:DV, :], rec_in[:], mult)
                    nc.sync.dma_start(
                        out=multiT[pr // 2][64:128, pr % 2, j0 : j0 + 512],
                        in_=tmp64[:],
                    )

        def attention(hooks_by_sweep, after_pair=None):
            """Two j-half sweeps over all pairs. Halving the attnV accumulators
            to [65,512] (1 bank each, 2 live) frees PSUM for a 6-deep scores
            ring: a scores matmul waits on an exp from ~3 i-chunks back, so
            projection hooks and LDW/copy latencies no longer starve the exp
            stream. Hooks and the lag-3 attnV are emitted BEFORE each
            iteration's scores to fill that wait. hooks_by_sweep[s][pr][ic]
            is a list of thunks."""
            from collections import deque

            pending = deque()   # (eTs, ic, pr, acc, sweep), newest at right

            def do_attnv(peT, pic, ppr, pacc, psweep):
                for hh in range(2):
                    mm(
                        pacc[hh][:],
                        v_sb[:, pic, 2 * ppr + hh, :],
                        peT[hh][:],
                        start=(pic == 0),
                        stop=(pic == NIC - 1),
                    )

            for pr in range(NPAIR):
                for sweep in range(2):
                    j0 = sweep * 512
                    acc = [
                        psAcc.tile(
                            [DV + 1, 512], F32, tag="acc", name=f"acc{sweep}_{pr}_{hh}"
                        )
                        for hh in range(2)
                    ]
                    hooks = hooks_by_sweep.get((pr, sweep), {})
                    # pair 0 runs a deeper attnV lag so its second j-half can
                    # carry the V projections for i-chunks 9-15, each hooked
                    # two iterations before its attnV pops
                    limit = 7 if pr == 0 else 3
                    for ic in range(NIC):
                        for fn in hooks.get(ic, ()):
                            fn()
                        while len(pending) >= limit:
                            pa = pending.popleft()
                            do_attnv(*pa)
                            if pa[1] == NIC - 1:
                                normalize(pa[2], pa[3], pa[4])
                        # scores^T, row-tiled pair (K=64 at partitions 0/64);
                        # one 1-bank PSUM slot and one exp per head
                        eTs = []
                        for hh in range(2):
                            ps_s = psS.tile(
                                [128, 512], F32, tag="sc",
                                name=f"ps_s{sweep}_{pr}_{ic}_{hh}",
                            )
                            mm(
                                ps_s[:],
                                qT_sb[hh * 64 : (hh + 1) * 64, pr, ic * 128 : (ic + 1) * 128],
                                kT_sb[hh * 64 : (hh + 1) * 64, pr, j0 : j0 + 512],
                                start=True,
                                stop=True,
                            )
                            eT = pe_pool.tile(
                                [128, 512], BF, tag="eT",
                                name=f"eT{sweep}_{pr}_{ic}_{hh}",
                            )
                            # x256-scaled q and k: fold 2^-16 into the exp scale
                            nc.scalar.activation(
                                eT[:], ps_s[:], Exp, scale=0.125 / (WSCALE * WSCALE)
                            )
                            eTs.append(eT)
                        pending.append((eTs, ic, pr, acc, sweep))
                    if after_pair and (sweep, pr) in after_pair:
                        after_pair[(sweep, pr)]()
            while pending:
                pa = pending.popleft()
                do_attnv(*pa)
                if pa[1] == NIC - 1:
                    normalize(pa[2], pa[3], pa[4])

        # Hook schedule for pair-major order (pair p: j-half-0 block then
        # j-half-1 block). Each pair's j-half-1 block has no inherent hook
        # needs, so it carries the NEXT pair's q t0 / k chains and a share of
        # the deferred V projections — load stays near-uniform across all 256
        # iterations instead of crowding a single global sweep.
        hooks = {}

        def add_hook(pr, sweep, ic, fn):
            hooks.setdefault((pr, sweep), {}).setdefault(ic, []).append(fn)

        # V heads 0-7 (pairs 0-3): i-chunks 0-9 in pair 0's first block, the
        # lag-6 tail (i-chunks 10-15) in its second block just before each is
        # consumed. V heads 8-15 (first used at pair 4) spread over pairs
        # 1-3's second blocks.
        add_hook(0, 0, 1, lambda: proj_v(0, 0))
        add_hook(0, 0, 1, lambda: proj_v(1, 0))
        for ic in range(2, 9):
            add_hook(0, 0, ic, lambda ic=ic: proj_v(ic, 0))
        add_hook(0, 0, 14, lambda: proj_v(9, 0))
        add_hook(0, 0, 15, lambda: proj_v(10, 0))
        for k in range(5):
            add_hook(0, 1, k, lambda k=k: proj_v(11 + k, 0))
        for k, ic in enumerate((1, 3, 5, 13, 14, 15)):
            add_hook(1, 1, ic, lambda k=k: proj_v(k, 1))
        for k, ic in enumerate((1, 2, 3, 13, 14)):
            add_hook(2, 1, ic, lambda k=k: proj_v(6 + k, 1))
        for k, ic in enumerate((1, 2, 3)):
            add_hook(3, 1, ic, lambda k=k: proj_v(11 + k, 1))
        add_hook(4, 0, 12, lambda: proj_v(14, 1))
        add_hook(4, 0, 13, lambda: proj_v(15, 1))
        for pr in range(NPAIR):
            # own q t1-3, consumed by this pair's first block from ic4/8/12 on
            for t, ic in ((1, 3), (2, 7), (3, 11)):
                add_hook(pr, 0, ic, lambda pr=pr, t=t: proj_chain(pr, "q", t))
        add_hook(0, 0, 5, lambda: proj_chain(0, "k", 1))
        for pr in range(7):  # next pair's bootstrap rides the j-half-1 block
            add_hook(pr, 1, 7, lambda pr=pr: proj_chain(pr + 1, "q", 0))
            add_hook(pr, 1, 9, lambda pr=pr: proj_chain(pr + 1, "k", 0))
            add_hook(pr, 1, 11, lambda pr=pr: proj_chain(pr + 1, "k", 1))

        pc_tiles = {}

        def open_phase_c():
            # pa's tensors are all dead once sweep 0 ends; reuse the space for
            # phase C inputs so their DMA overlaps all of sweep 1.
            pa.release()
            pc = stack.enter_context(tc.tile_pool(name="pc", bufs=1))
            pc_tiles["wo"] = pc.tile([128, NKC, D], F8, name="wo_sb")
            pc_tiles["tokres"] = pc.tile([128, NJCH, D], F32, name="tokres_sb")
            nc.sync.dma_start(pc_tiles["wo"][:], wo_d[:])
            nc.sync.dma_start(pc_tiles["tokres"][:], tokres_d[:])

        # upfront: just enough projection for sweep-0 pair-0's first scores
        proj_chain(0, "q", 0)
        proj_chain(0, "k", 0)

        attention(hooks, after_pair={(0, 7): open_phase_c})
        wo_sb = pc_tiles["wo"]
        tokres_sb = pc_tiles["tokres"]
        paJ.release()
        psAcc.release()
        psS.release()
        # ---------------- Phase C: out-proj + residual + LayerNorm ----------------
        with (
            tc.tile_pool(name="pC", bufs=4) as pC,
            tc.tile_pool(name="pStats", bufs=8) as pStats,
            tc.tile_pool(name="psC", bufs=4, space="PSUM") as psC,
        ):
            # Out-proj in two steps per jch: kc 0-6 accumulate early (their
            # multiT chunks are ready pairs before the last normalize), kc 7
            # finishes when multiT[7] lands. Prefilling 4 PSUM groups hides
            # the last normalize's DRAM round-trip behind ~12us of matmuls.
            prefill = {}

            def emit_prefill(jch):
                ps_o = psC.tile([128, D], F32, tag="po", name=f"ps_o{jch}")
                for kcp in range(NKC // 2 - 1):
                    lhsT = multiT[kcp][:, :, jch * 128 : (jch + 1) * 128]
                    for nb in range(2):
                        mm(
                            ps_o[:, nb * 512 : (nb + 1) * 512],
                            lhsT,
                            wo_sb[:, 2 * kcp : 2 * kcp + 2, nb * 512 : (nb + 1) * 512],
                            start=(kcp == 0),
                            stop=False,
                            perf_mode=DR,
                        )
                prefill[jch] = ps_o

            for jch in range(4):
                emit_prefill(jch)
            for jch in range(NJCH):
                ps_o = prefill.pop(jch)
                lhsT = multiT[NKC // 2 - 1][:, :, jch * 128 : (jch + 1) * 128]
                for nb in range(2):
                    mm(
                        ps_o[:, nb * 512 : (nb + 1) * 512],
                        lhsT,
                        wo_sb[:, NKC - 2 : NKC, nb * 512 : (nb + 1) * 512],
                        start=False,
                        stop=True,
                        perf_mode=DR,
                    )
                # x = psum + residual, sum_t = rowsum(x), in one DVE pass
                x_sb = pC.tile([128, D], F32, tag="x", name=f"x{jch}")
                sum_t = pStats.tile([128, 1], F32, tag="sum", name=f"sum{jch}")
                # x = psum/WSCALE + residual (undo the Wo fp8 pre-scale),
                # sum_t = rowsum(x), in one DVE pass
                nc.vector.scalar_tensor_tensor(
                    out=x_sb[:],
                    in0=ps_o[:],
                    scalar=1.0 / WSCALE,
                    in1=tokres_sb[:, jch, :],
                    op0=mult,
                    op1=add,
                    accum_out=sum_t[:],
                )
                negmean = pStats.tile([128, 1], F32, tag="nm", name=f"nm{jch}")
                nc.vector.tensor_scalar_mul(negmean[:], sum_t[:], -1.0 / D)
                # Variance: alternate engines by jch parity so the serialized
                # LN tail splits between Scalar (sum((x-m)^2) via Square) and
                # DVE (sum(x^2)/D - mean^2 via an STT x*x pass).
                scrap = pC.tile([128, D], BF, tag="scrap", name=f"scrap{jch}")
                ssq = pStats.tile([128, 1], F32, tag="ssq", name=f"ssq{jch}")
                if jch % 2 == 0:
                    nc.scalar.activation(
                        scrap[:], x_sb[:], Square, bias=negmean[:], accum_out=ssq[:]
                    )
                else:
                    nc.vector.scalar_tensor_tensor(
                        out=scrap[:], in0=x_sb[:], scalar=0.0, in1=x_sb[:],
                        op0=add, op1=mult, accum_out=ssq[:],
                    )
                    m2 = pStats.tile([128, 1], F32, tag="m2", name=f"m2{jch}")
                    nc.vector.tensor_tensor(m2[:], negmean[:], negmean[:], mult)
                    nc.vector.tensor_scalar_mul(m2[:], m2[:], -float(D))
                    nc.vector.tensor_tensor(ssq[:], ssq[:], m2[:], add)
                std_t = pStats.tile([128, 1], F32, tag="std", name=f"std{jch}")
                nc.scalar.activation(std_t[:], ssq[:], Sqrt, bias=eps_sb[:], scale=1.0 / D)
                rstd = pStats.tile([128, 1], F32, tag="rstd", name=f"rstd{jch}")
                nc.vector.reciprocal(rstd[:], std_t[:])
                # (x - m) * rstd == x*rstd + (negmean*rstd), one ACT op
                rstd_nm = pStats.tile([128, 1], F32, tag="rnm", name=f"rnm{jch}")
                nc.vector.tensor_tensor(rstd_nm[:], negmean[:], rstd[:], mult)
                out_sb = pC.tile([128, D], F32, tag="out", name=f"out{jch}")
                nc.scalar.activation(
                    out_sb[:],
                    x_sb[:],
                    mybir.ActivationFunctionType.Identity,
                    bias=rstd_nm[:],
                    scale=rstd[:],
                )
                if apply_affine:
                    nc.gpsimd.tensor_tensor(out_sb[:], out_sb[:], gamma_sb[:], mult)
                    nc.gpsimd.tensor_tensor(out_sb[:], out_sb[:], beta_sb[:], add)
                nc.sync.dma_start(out_d[:, jch], out_sb[:])
                # second prefill wave once the first four STTs are emitted, so
                # the PE chain never parks on a not-yet-freed PSUM group
                if jch == 3:
                    for j2 in range(4, NJCH):
                        emit_prefill(j2)

    nc.compile()
    return nc


def _prep_inputs(tokens, Wq, Wk, Wv, Wo, gamma, beta):
    """Host-side layout prep. Returns per-core input maps."""
    tokens = np.ascontiguousarray(np.asarray(tokens, dtype=np.float32))
    # weights -> [p, kc, n] with row index kc*128+p
    def rows128(a):  # [1024, N] -> [128, 8, N]
        return np.ascontiguousarray(
            a.reshape(NKC, 128, a.shape[-1]).transpose(1, 0, 2)
        )

    wq_all = rows128(
        (np.asarray(Wq).transpose(1, 0, 2).reshape(D, H * DK) * WSCALE).astype(FP8)
    )
    wk_all = rows128(
        (np.asarray(Wk).transpose(1, 0, 2).reshape(D, H * DK) * WSCALE).astype(FP8)
    )
    wv_all = rows128(
        (np.asarray(Wv).transpose(1, 0, 2).reshape(D, H * DV) * WSCALE).astype(FP8)
    )
    wo_all = rows128((np.asarray(Wo) * WSCALE).astype(FP8))
    gamma_bc = np.ascontiguousarray(
        np.broadcast_to(np.asarray(gamma, np.float32), (128, D))
    )
    beta_bc = np.ascontiguousarray(
        np.broadcast_to(np.asarray(beta, np.float32), (128, D))
    )

    tokT_by_b = []
    for b in range(B):
        tokT_by_b.append(rows128(tokens[b].T.astype(FP8)))  # [128, 8, 2048]

    in_maps = []
    for c in range(NCORES):
        b, jc = c // 2, c % 2
        tokT = tokT_by_b[b]
        tokTj = np.ascontiguousarray(tokT[:, :, jc * JW : (jc + 1) * JW])
        tokres = np.ascontiguousarray(
            tokens[b, jc * JW : (jc + 1) * JW]
            .reshape(NJCH, 128, D)
            .transpose(1, 0, 2)
        )
        in_maps.append(
            {
                "tokT": tokT,
                "tokTj": tokTj,
                "wq": wq_all,
                "wk": wk_all,
                "wv": wv_all,
                "wo": wo_all,
                "tokres": tokres,
                "gamma_bc": gamma_bc,
                "beta_bc": beta_bc,
            }
        )
    return in_maps


def run(inputs, trace=False, tmpdir=None):
    """Run on hardware; returns (output, BassKernelResults)."""
    from concourse.bass_utils import run_bass_kernel_spmd

    apply_affine = not (
        np.all(np.asarray(inputs["gamma"]) == 1.0)
        and np.all(np.asarray(inputs["beta"]) == 0.0)
    )
    key = ("nc", apply_affine)
    if key not in _CACHE:
        _CACHE[key] = _build_nc(apply_affine)
    nc = _CACHE[key]
    in_maps = _prep_inputs(**inputs)
    res = run_bass_kernel_spmd(
        nc, in_maps, core_ids=list(range(NCORES)), trace=trace, tmpdir=tmpdir
    )
    out = np.empty((B, S, D), np.float32)
    for c in range(NCORES):
        b, jc = c // 2, c % 2
        o = res.results[c]["out"]  # [128, 8, 1024]
        out[b, jc * JW : (jc + 1) * JW] = (
            o.transpose(1, 0, 2).reshape(JW, D)
        )
    return out, res


def kernel(tokens, Wq, Wk, Wv, Wo, gamma, beta):
    out, _ = run(
        dict(tokens=tokens, Wq=Wq, Wk=Wk, Wv=Wv, Wo=Wo, gamma=gamma, beta=beta)
    )
    return out
